# revision 17
# baseline (speedup 1.0000x reference)
"""BioTripletLoss Trainium2 kernel.

Data-parallel over the batch dim across 8 NeuronCores. Each core:
  - gets a 2048-row shard of h, r, t
  - gets a full replicated copy of t (for the global neg_idx gather,
    done on-device with indirect DMA)
  - computes per-sample losses for its shard; host averages.

Layout: shard rows are processed as 4 super-tiles of [128 partitions x
4096 floats] (4 consecutive rows per partition => 16 KiB contiguous DRAM
per partition => 2 MiB DMAs). Stat column c = 4*i + j corresponds to
shard row 512*i + 4*p + j at partition p.
"""

import math

import numpy as np

import concourse.bacc as bacc
import concourse.bass as bass
import concourse.tile as tile
from concourse import mybir
from concourse.bass_utils import run_bass_kernel_spmd

B = 16384
D = 1024
N_CORES = 8
SH = B // N_CORES          # 2048 rows per core
P = 128                    # partitions
RPP = 2                    # rows per partition per super-tile
ST = SH // (P * RPP)       # 4 super-tiles
COLS = ST * RPP            # 16 stat columns
W = RPP * D                # 4096 floats per partition per super-tile

MARGIN = 0.3
MIN_POS_DIST = 0.1
PUSH_SCALE = 2.0

F32 = mybir.dt.float32
I32 = mybir.dt.int32

_PROG = None  # (nc, input names) cache


def _build_program():
    nc = bacc.Bacc(
        "TRN2",
        target_bir_lowering=False,
        debug=False,
        num_devices=N_CORES,
        num_swdge_queues=4,
    )

    h = nc.dram_tensor("h_s", [SH, D], F32, kind="ExternalInput").ap()
    r = nc.dram_tensor("r_s", [SH, D], F32, kind="ExternalInput").ap()
    t = nc.dram_tensor("t_s", [SH, D], F32, kind="ExternalInput").ap()
    tf = nc.dram_tensor("t_full", [B, D], F32, kind="ExternalInput").ap()
    # int16 gather indices, SWDGE dma_gather layout (see _make_gather_idx)
    ni = nc.dram_tensor(
        "neg_l", [P, ST * (P * RPP // 16)], mybir.dt.int16, kind="ExternalInput"
    ).ap()
    mk = nc.dram_tensor("mask_l", [P, COLS], F32, kind="ExternalInput").ap()
    out = nc.dram_tensor("loss_l", [P, COLS], F32, kind="ExternalOutput").ap()

    AF = mybir.ActivationFunctionType
    OP = mybir.AluOpType

    with tile.TileContext(nc) as tc:
        with (
            tc.tile_pool(name="io", bufs=1) as iop,
            tc.tile_pool(name="stream", bufs=3) as sp,
            tc.tile_pool(name="scr", bufs=3) as scp,
            tc.tile_pool(name="tail", bufs=1) as tp,
        ):
            idx_cols = ST * (P * RPP // 16)  # 32 idx columns per super-tile
            ni_sb = iop.tile([P, idx_cols], mybir.dt.int16)
            nc.sync.dma_start(out=ni_sb[:], in_=ni)
            mk_sb = iop.tile([P, COLS], F32)
            nc.sync.dma_start(out=mk_sb[:], in_=mk)
            pos_sq = iop.tile([P, COLS], F32)
            neg_sq = iop.tile([P, COLS], F32)

            for i in range(ST):
                rows = slice(i * P * RPP, (i + 1) * P * RPP)
                h_t = sp.tile([P, W], F32, tag="h")
                r_t = sp.tile([P, W], F32, tag="r")
                t_t = sp.tile([P, W], F32, tag="t")
                n_t = sp.tile([P, W], F32, tag="n")
                hr_t = sp.tile([P, W], F32, tag="hr")

                nc.sync.dma_start(
                    out=h_t[:], in_=h[rows, :].rearrange("(p q) d -> p (q d)", p=P)
                )
                nc.scalar.dma_start(
                    out=r_t[:], in_=r[rows, :].rearrange("(p q) d -> p (q d)", p=P)
                )
                nc.sync.dma_start(
                    out=t_t[:], in_=t[rows, :].rearrange("(p q) d -> p (q d)", p=P)
                )
                # gather 512 rows of t in one SWDGE dma_gather; row g of the
                # gather lands at out[g % 128, g // 128, :], matching the
                # (partition, sub-row) layout of the direct h/r/t loads.
                nidx = P * RPP  # 512
                nc.gpsimd.dma_gather(
                    out_ap=n_t[:].rearrange("p (c d) -> p c d", d=D),
                    in_ap=tf,
                    idxs_ap=ni_sb[:, i * (nidx // 16) : (i + 1) * (nidx // 16)],
                    num_idxs=nidx,
                    num_idxs_reg=nidx,
                    elem_size=D,
                    queue_num=i % 4,
                )

                # hr = h + r ; d0 = hr - t (in place on t) ; d1 = hr - tneg
                nc.vector.tensor_tensor(
                    out=hr_t[:], in0=h_t[:], in1=r_t[:], op=OP.add
                )
                nc.vector.tensor_tensor(
                    out=t_t[:], in0=hr_t[:], in1=t_t[:], op=OP.subtract
                )
                nc.vector.tensor_tensor(
                    out=n_t[:], in0=hr_t[:], in1=n_t[:], op=OP.subtract
                )

                for j in range(RPP):
                    c = i * RPP + j
                    scr0 = scp.tile([P, D], F32, tag="scr")
                    nc.scalar.activation(
                        out=scr0[:],
                        in_=t_t[:, j * D : (j + 1) * D],
                        func=AF.Square,
                        accum_out=pos_sq[:, c : c + 1],
                    )
                    scr1 = scp.tile([P, D], F32, tag="scr")
                    nc.scalar.activation(
                        out=scr1[:],
                        in_=n_t[:, j * D : (j + 1) * D],
                        func=AF.Square,
                        accum_out=neg_sq[:, c : c + 1],
                    )

            # ---- tail: per-sample loss on [P, COLS] ----
            def bias_ap(val, _n=[0]):
                _n[0] += 1
                b = tp.tile([P, 1], F32, tag=f"bias{_n[0]}")
                nc.vector.memset(b[:], val)
                return b[:]

            b_margin = bias_ap(MARGIN)
            b_minpos = bias_ap(0.3 * MIN_POS_DIST)
            b_currm = bias_ap(MARGIN * PUSH_SCALE)
            b_lnhalf = bias_ap(math.log(0.5))
            b_zero = bias_ap(0.0)

            pos = tp.tile([P, COLS], F32)
            nc.scalar.activation(out=pos[:], in_=pos_sq[:], func=AF.Sqrt, bias=b_zero)
            neg = tp.tile([P, COLS], F32)
            nc.scalar.activation(out=neg[:], in_=neg_sq[:], func=AF.Sqrt, bias=b_zero)

            # loss_sim = relu(pos - neg + MARGIN) + 0.3*relu(MIN_POS_DIST - pos)
            diff = tp.tile([P, COLS], F32)
            nc.vector.tensor_tensor(
                out=diff[:], in0=pos[:], in1=neg[:], op=OP.subtract
            )
            relu1 = tp.tile([P, COLS], F32)
            nc.scalar.activation(
                out=relu1[:], in_=diff[:], func=AF.Relu, bias=b_margin
            )
            # 0.3*relu(0.1 - pos) == relu(0.03 - 0.3*pos)
            relu2 = tp.tile([P, COLS], F32)
            nc.scalar.activation(
                out=relu2[:],
                in_=pos[:],
                func=AF.Relu,
                scale=-0.3,
                bias=b_minpos,
            )
            ls = tp.tile([P, COLS], F32)
            nc.vector.tensor_tensor(out=ls[:], in0=relu1[:], in1=relu2[:], op=OP.add)

            # loss_dissim = relu(0.6 - pos) + 0.5*exp(-pos)
            relu3 = tp.tile([P, COLS], F32)
            nc.scalar.activation(
                out=relu3[:],
                in_=pos[:],
                func=AF.Relu,
                scale=-1.0,
                bias=b_currm,
            )
            expt = tp.tile([P, COLS], F32)
            nc.scalar.activation(
                out=expt[:], in_=pos[:], func=AF.Exp, scale=-1.0, bias=b_lnhalf
            )
            ld = tp.tile([P, COLS], F32)
            nc.vector.tensor_tensor(out=ld[:], in0=relu3[:], in1=expt[:], op=OP.add)

            # per = ls + mask * (ld - ls)
            dmd = tp.tile([P, COLS], F32)
            nc.vector.tensor_tensor(out=dmd[:], in0=ld[:], in1=ls[:], op=OP.subtract)
            dmm = tp.tile([P, COLS], F32)
            nc.vector.tensor_tensor(out=dmm[:], in0=dmd[:], in1=mk_sb[:], op=OP.mult)
            per = tp.tile([P, COLS], F32)
            nc.vector.tensor_tensor(out=per[:], in0=ls[:], in1=dmm[:], op=OP.add)

            nc.sync.dma_start(out=out, in_=per[:])

    nc.finalize()
    return nc


def _get_program():
    global _PROG
    if _PROG is None:
        _PROG = _build_program()
    return _PROG


def _to_layout(x):
    """shard [SH] -> [P, COLS] with layout[p, 4i+j] = x[512*i + 4*p + j]."""
    return np.ascontiguousarray(
        x.reshape(ST, P, RPP).transpose(1, 0, 2).reshape(P, COLS)
    )


def _from_layout(y):
    """[P, COLS] -> shard [SH] (inverse of _to_layout)."""
    return y.reshape(P, ST, RPP).transpose(1, 0, 2).reshape(SH)


def _make_gather_idx(neg_shard):
    """Build the SWDGE dma_gather int16 index tile [P, ST*32].

    For super-tile i, linear gather slot g in [0, 512) lands at SBUF
    (partition g%128, sub-row g//128); we want that slot to hold
    t[neg_shard[512*i + 4*(g%128) + g//128]]. dma_gather reads its index
    list wrapped over 16 partitions: idx[p16, s] = linear[s*16 + p16].
    """
    nidx = P * RPP
    scols = nidx // 16
    out = np.zeros((P, ST * scols), dtype=np.int16)
    g = np.arange(nidx)
    for i in range(ST):
        lin = neg_shard[nidx * i + RPP * (g % P) + g // P].astype(np.int16)
        # replicate the 16-partition wrap into all 8 gpsimd-core groups
        out[:, i * scols : (i + 1) * scols] = np.tile(
            lin.reshape(scols, 16).T, (P // 16, 1)
        )
    return out


def _make_in_maps(h, t, r, relation_ids, neg_idx):
    h = np.ascontiguousarray(h, dtype=np.float32)
    t = np.ascontiguousarray(t, dtype=np.float32)
    r = np.ascontiguousarray(r, dtype=np.float32)
    neg = np.asarray(neg_idx).astype(np.int32)
    mask = (np.asarray(relation_ids) == 1).astype(np.float32)

    in_maps = []
    for k in range(N_CORES):
        rows = slice(k * SH, (k + 1) * SH)
        in_maps.append(
            {
                "h_s": np.ascontiguousarray(h[rows]),
                "r_s": np.ascontiguousarray(r[rows]),
                "t_s": np.ascontiguousarray(t[rows]),
                "t_full": t,
                "neg_l": _make_gather_idx(neg[rows]),
                "mask_l": _to_layout(mask[rows]),
            }
        )
    return in_maps


def _postprocess(results):
    per_sample = np.concatenate(
        [_from_layout(res["loss_l"]) for res in results]
    )
    return np.float32(per_sample.astype(np.float64).mean())


def kernel(h, t, r, relation_ids, neg_idx):
    nc = _get_program()
    in_maps = _make_in_maps(h, t, r, relation_ids, neg_idx)
    res = run_bass_kernel_spmd(nc, in_maps, core_ids=list(range(N_CORES)))
    return _postprocess(res.results)


def _ensure_ntff_hook():
    """Register antenv.axon_hooks if the agent image lacks it, using the
    same ctypes NTFF mechanism trn_boot would have installed."""
    try:
        from antenv.axon_hooks import get_axon_ntff_profile_hook  # noqa: F401

        return
    except ImportError:
        pass
    import sys
    import types

    import antenv
    from trn_agent_boot.trn_boot import _ntff_profile_via_ctypes

    hook = _ntff_profile_via_ctypes("/opt/axon/libaxon_pjrt.so")
    mod = types.ModuleType("antenv.axon_hooks")
    mod.get_axon_ntff_profile_hook = lambda: hook
    mod.set_axon_ntff_profile_hook = lambda h: None
    sys.modules["antenv.axon_hooks"] = mod
    antenv.axon_hooks = mod


def run_traced(h, t, r, relation_ids, neg_idx):
    """Like kernel(), but returns (output, exec_time_ns, trace_path)."""
    _ensure_ntff_hook()
    nc = _get_program()
    in_maps = _make_in_maps(h, t, r, relation_ids, neg_idx)
    res = run_bass_kernel_spmd(
        nc, in_maps, core_ids=list(range(N_CORES)), trace=True
    )
    trace_path = None
    if res.instructions_and_trace is not None:
        trace_path = res.instructions_and_trace[1]
    return _postprocess(res.results), res.exec_time_ns, trace_path
